# revision 1
# baseline (speedup 1.0000x reference)
"""Trainium2 kernel for nn_IteratedLinearNet: y = x @ (W.T)^60.

Strategy (8 NeuronCores, single SPMD launch):
  - matrix power by squaring via the addition chain 2, 4, 8, 12, 24, 48, 60
    (7 matmuls of 2048^3 instead of 60 applications of x @ W.T)
  - each product is tensor-sharded: core j computes a 256-wide column slab
  - after each product (except the last) the core transposes its slab on
    TensorE and an 8-core AllGather assembles the full transposed matrix,
    which is the next product's stationary operand; AllGathers are split
    into column halves so compute pipelines with communication
  - final apply is tensor-parallel: core j computes y[:, Sj] for the full
    batch with x.T streamed from HBM
  - all matmuls run in float32r (FP22-truncated reads, full PE rate);
    inputs are pre-rounded to FP22-nearest on the host to keep the
    truncation exact and unbiased

Self-contained: builds/compiles on first call and caches the module.
"""

import numpy as np

_GRID = 2048
_BATCH = 4096
_NCORES = 8
_SW = _GRID // _NCORES  # 256
_KT = _GRID // 128  # 16
_HALF = _GRID // 2

# (power, lhsT_src, rhs_buf, out_buf); lhsT_src: "wt" or index of the step
# whose AllGather output (the transposed full matrix) is the stationary side.
_CHAIN = [
    (2, "wt", 0, 1),
    (4, 0, 1, 2),
    (8, 1, 2, 0),
    (12, 2, 2, 0),  # A12 = A8 @ A4 (rhs = A4 slab, still in buf 2)
    (24, 3, 0, 1),
    (48, 4, 1, 2),
    (60, 5, 0, 1),
]

_cache = {}


def _build():
    from contextlib import ExitStack

    import concourse.tile as tile
    from concourse import bacc, masks, mybir

    F32R = mybir.dt.float32r
    F32 = mybir.dt.float32
    G, KT, SW, HALF, BATCH = _GRID, _KT, _SW, _HALF, _BATCH

    nc = bacc.Bacc(None, target_bir_lowering=False, num_devices=_NCORES)
    wt = nc.declare_dram_parameter("wt", [G, G], F32R, isOutput=False)
    aslab = nc.declare_dram_parameter("aslab", [G, SW], F32R, isOutput=False)
    xt = nc.declare_dram_parameter("xt", [G, BATCH], F32R, isOutput=False)
    ytj = nc.declare_dram_parameter("ytj", [SW, BATCH], F32R, isOutput=True)

    rg = [list(range(_NCORES))]

    with ExitStack() as ctx:
        tc = ctx.enter_context(tile.TileContext(nc))
        big = ctx.enter_context(tc.tile_pool(name="big", bufs=1))
        slabs = ctx.enter_context(tc.tile_pool(name="slabs", bufs=1))
        shpool = ctx.enter_context(tc.tile_pool(name="shpool", bufs=3))
        ypool = ctx.enter_context(tc.tile_pool(name="ypool", bufs=2))
        mmps = ctx.enter_context(tc.tile_pool(name="mmps", bufs=4, space="PSUM"))
        tps = ctx.enter_context(tc.tile_pool(name="tps", bufs=2, space="PSUM"))
        dram = ctx.enter_context(tc.tile_pool(name="dram", bufs=2, space="DRAM"))

        lhsT_sb = big.tile([128, KT, G], F32R)
        sbuf = [
            slabs.tile([128, KT, SW], F32R, name=f"slab{i}", tag=f"slab{i}")
            for i in range(3)
        ]
        ident32 = slabs.tile([128, 128], F32, name="ident32", tag="ident32")
        masks.make_identity(nc, ident32[:])
        ident = slabs.tile([128, 128], F32R, name="ident", tag="ident")
        nc.vector.tensor_copy(ident[:], ident32[:])

        for k in range(KT):
            nc.sync.dma_start(sbuf[0][:, k, :], aslab[128 * k : 128 * (k + 1), :])

        ag_outs = []
        n_steps = len(_CHAIN)
        for si, (power, src, rb, ob) in enumerate(_CHAIN):
            is_last = si == n_steps - 1
            rhs = sbuf[rb]
            out = sbuf[ob]
            ag_out_halves = []
            for h in range(2):
                for k in range(KT):
                    if src == "wt":
                        s_ap = wt[128 * k : 128 * (k + 1), HALF * h : HALF * (h + 1)]
                    else:
                        s_ap = ag_outs[src][h][128 * k : 128 * (k + 1), :]
                    nc.sync.dma_start(lhsT_sb[:, k, HALF * h : HALF * (h + 1)], s_ap)
                for m in range(8 * h, 8 * h + 8):
                    ps = mmps.tile([128, SW], F32, name="ps", tag="ps")
                    for k in range(KT):
                        nc.tensor.matmul(
                            ps[:],
                            lhsT_sb[:, k, 128 * m : 128 * (m + 1)],
                            rhs[:, k, :],
                            start=(k == 0),
                            stop=(k == KT - 1),
                        )
                    nc.vector.tensor_copy(out[:, m, :], ps[:])
                if is_last:
                    continue
                t_sb = shpool.tile([128, 2, HALF], F32R, name=f"t{si}_{h}", tag="sh8")
                for k in range(8 * h, 8 * h + 8):
                    for a in range(2):
                        psT = tps.tile([128, 128], F32R, name="psT", tag="psT")
                        nc.tensor.transpose(
                            psT[:], out[:, k, 128 * a : 128 * (a + 1)], ident[:]
                        )
                        nc.vector.tensor_copy(
                            t_sb[:, a, 128 * (k - 8 * h) : 128 * (k - 8 * h + 1)],
                            psT[:],
                        )
                ag_in = dram.tile([SW, HALF], F32R, name=f"agin{si}_{h}", tag="agin")
                for a in range(2):
                    nc.sync.dma_start(ag_in[128 * a : 128 * (a + 1), :], t_sb[:, a, :])
                ag_out = dram.tile(
                    [G, HALF],
                    F32R,
                    name=f"agout{si}_{h}",
                    tag="agout",
                    addr_space="Shared",
                )
                nc.gpsimd.collective_compute(
                    "AllGather",
                    mybir.AluOpType.bypass,
                    replica_groups=rg,
                    ins=[ag_in.opt()],
                    outs=[ag_out.opt()],
                )
                ag_out_halves.append(ag_out)
            ag_outs.append(ag_out_halves)

        final = sbuf[_CHAIN[-1][3]]
        for c in range(BATCH // SW):
            pss = [
                mmps.tile([128, SW], F32, name=f"psy{a}", tag="ps") for a in range(2)
            ]
            for kh in range(2):
                xchunk = shpool.tile([128, KT // 2, SW], F32R, name="xchunk", tag="sh8")
                for kk in range(KT // 2):
                    k = 8 * kh + kk
                    nc.sync.dma_start(
                        xchunk[:, kk, :],
                        xt[128 * k : 128 * (k + 1), SW * c : SW * (c + 1)],
                    )
                for a in range(2):
                    for kk in range(KT // 2):
                        k = 8 * kh + kk
                        nc.tensor.matmul(
                            pss[a][:],
                            final[:, k, 128 * a : 128 * (a + 1)],
                            xchunk[:, kk, :],
                            start=(k == 0),
                            stop=(k == KT - 1),
                        )
            for a in range(2):
                ystage = ypool.tile([128, SW], F32R, name="ystage", tag="ystage")
                nc.vector.tensor_copy(ystage[:], pss[a][:])
                nc.sync.dma_start(
                    ytj[128 * a : 128 * (a + 1), SW * c : SW * (c + 1)], ystage[:]
                )
    nc.compile()
    return nc


def _round22(a):
    bits = np.ascontiguousarray(a).view(np.uint32)
    return ((bits + 0x200) & np.uint32(0xFFFFFC00)).view(np.float32)


def kernel(x, W):
    from concourse.bass_utils import run_bass_kernel_spmd

    if "nc" not in _cache:
        _cache["nc"] = _build()
    nc = _cache["nc"]

    Wr = _round22(np.asarray(W, dtype=np.float32))
    xr = _round22(np.asarray(x, dtype=np.float32))
    wt_np = np.ascontiguousarray(Wr)
    xt_np = np.ascontiguousarray(xr.T)
    in_maps = [
        {
            "wt": wt_np,
            "aslab": np.ascontiguousarray(Wr[_SW * j : _SW * (j + 1), :].T),
            "xt": xt_np,
        }
        for j in range(_NCORES)
    ]
    res = run_bass_kernel_spmd(nc, in_maps, core_ids=list(range(_NCORES)))
    _cache["last_exec_time_ns"] = res.exec_time_ns
    _cache["last_results"] = res
    y = np.concatenate(
        [res.results[j]["ytj"].T for j in range(_NCORES)], axis=1
    ).astype(np.float32)
    return y



# revision 3
# speedup vs baseline: 1.7641x; 1.7641x over previous
"""Trainium2 kernel for nn_IteratedLinearNet: y = x @ (W.T)^60.

Strategy (8 NeuronCores, single SPMD launch):
  - A' = c*W.T with c = 1/spectral_radius(W.T) estimated host-side, so all
    powers of A' stay O(1) and the whole chain runs in float16 (10-bit
    mantissa) at full PE rate with half the DMA/collective bytes of fp32;
    the c^-60 factor is undone on the host.
  - row-slab matrix-power chain: core j keeps the transposed row-slab
    (C_a[Sj,:])^T as the stationary operand and multiplies by a REPLICATED
    full matrix C_b: C_{a+b}[Sj,:] = C_a[Sj,:] @ C_b.  With the addition
    chain 2,4,8,12,24,48,60 only C2, C4, C12, C24, C60 are ever needed
    full, so just 5 AllGathers (vs 12 in the column-slab formulation).
  - gathered matrices stream from DRAM into SBUF K-chunk by K-chunk,
    overlapped with the consuming product's matmuls.
  - the final apply is batch-data-parallel: core j computes
    y[Bj,:] = x[Bj,:] @ C60 with the tiny x^T[:,Bj] slice as stationary.
  - a dummy warmup AllGather is triggered at kernel start so the one-time
    collective barrier/ncfw warmup overlaps the first product.

Self-contained: builds/compiles on first call and caches the module.
"""

import numpy as np

_G = 2048
_B = 4096
_NC = 8
_SW = _G // _NC  # 256 slab rows per core
_BW = _B // _NC  # 512 batch rows per core
_KT = _G // 128  # 16 K-chunks

_cache = {}


def _build():
    from contextlib import ExitStack

    import concourse.tile as tile
    from concourse import bacc, masks, mybir

    F16 = mybir.dt.float16
    F32 = mybir.dt.float32
    G, KT, SW, BW = _G, _KT, _SW, _BW

    nc = bacc.Bacc(None, target_bir_lowering=False, num_devices=_NC)
    afull = nc.declare_dram_parameter("afull", [G, G], F16, isOutput=False)
    slabt0 = nc.declare_dram_parameter("slabt0", [G, SW], F16, isOutput=False)
    xbj = nc.declare_dram_parameter("xbj", [G, BW], F16, isOutput=False)
    yj = nc.declare_dram_parameter("yj", [BW, G], F32, isOutput=True)

    rg = [list(range(_NC))]

    with ExitStack() as ctx:
        tc = ctx.enter_context(tile.TileContext(nc))
        fullp = ctx.enter_context(tc.tile_pool(name="fullp", bufs=2))
        stp = ctx.enter_context(tc.tile_pool(name="stp", bufs=2))
        ssp = ctx.enter_context(tc.tile_pool(name="ssp", bufs=2))
        misc = ctx.enter_context(tc.tile_pool(name="misc", bufs=1))
        ypool = ctx.enter_context(tc.tile_pool(name="ypool", bufs=4))
        mmps = ctx.enter_context(tc.tile_pool(name="mmps", bufs=4, space="PSUM"))
        tps = ctx.enter_context(tc.tile_pool(name="tps", bufs=2, space="PSUM"))
        dram = ctx.enter_context(tc.tile_pool(name="dram", bufs=2, space="DRAM"))

        ident32 = misc.tile([128, 128], F32, name="ident32", tag="ident32")
        masks.make_identity(nc, ident32[:])
        ident = misc.tile([128, 128], F16, name="ident", tag="ident")
        nc.vector.tensor_copy(ident[:], ident32[:])

        # warmup collective: absorbs the one-time barrier + ncfw ramp while
        # the first product (which needs no comm) runs.
        wtile = misc.tile([128, 16], F16, name="warm", tag="warm")
        nc.vector.tensor_copy(wtile[:], ident32[:, 0:16])
        win = dram.tile([128, 16], F16, name="warmin", tag="warmin")
        nc.sync.dma_start(win[:], wtile[:])
        wout = dram.tile(
            [128 * _NC, 16], F16, name="warmout", tag="warmout", addr_space="Shared"
        )
        nc.gpsimd.collective_compute(
            "AllGather",
            mybir.AluOpType.bypass,
            replica_groups=rg,
            ins=[win.opt()],
            outs=[wout.opt()],
        )

        # stationary slabT for step 1 + streamed A' + x^T slice prefetch
        st1 = stp.tile([128, KT, SW], F16, name="st1", tag="st")
        fA = fullp.tile([128, KT, G], F16, name="fA", tag="full")
        for k in range(KT):
            nc.sync.dma_start(st1[:, k, :], slabt0[128 * k : 128 * (k + 1), :])
            nc.sync.dma_start(fA[:, k, :], afull[128 * k : 128 * (k + 1), :])
        xsb = misc.tile([128, KT, BW], F16, name="xsb", tag="xsb")
        for k in range(KT):
            nc.sync.dma_start(xsb[:, k, :], xbj[128 * k : 128 * (k + 1), :])

        def product(st, F, reload_src, si):
            """slab = (st)^T-rows @ F, returns normal-form slab staging tile.

            st: [128, KT, SW] transposed slab (lhsT source)
            F:  [128, KT, G] full matrix tile; if reload_src is given, its
                K-chunks are DMA'd in here first (overlapping the matmuls).
            """
            if reload_src is not None:
                for k in range(KT):
                    nc.sync.dma_start(
                        F[:, k, :], reload_src[128 * k : 128 * (k + 1), :]
                    )
            ssb = ssp.tile([128, 2, G], F16, name=f"ss{si}", tag="ss")
            for rnd in range(2):
                pts = [
                    [
                        mmps.tile([128, 512], F32, name=f"p{si}_{rnd}_{mr}_{mci}", tag="mm")
                        for mci in range(2)
                    ]
                    for mr in range(2)
                ]
                for k in range(KT):
                    for mr in range(2):
                        for mci in range(2):
                            mc = 2 * rnd + mci
                            nc.tensor.matmul(
                                pts[mr][mci][:],
                                st[:, k, 128 * mr : 128 * (mr + 1)],
                                F[:, k, 512 * mc : 512 * (mc + 1)],
                                start=(k == 0),
                                stop=(k == KT - 1),
                            )
                for mr in range(2):
                    for mci in range(2):
                        mc = 2 * rnd + mci
                        nc.vector.tensor_copy(
                            ssb[:, mr, 512 * mc : 512 * (mc + 1)], pts[mr][mci][:]
                        )
            return ssb

        def do_ag(ssb, si):
            agin = dram.tile([SW, G], F16, name=f"agin{si}", tag="agin")
            for r in range(2):
                nc.sync.dma_start(agin[128 * r : 128 * (r + 1), :], ssb[:, r, :])
            agout = dram.tile(
                [G, G], F16, name=f"agout{si}", tag="agout", addr_space="Shared"
            )
            nc.gpsimd.collective_compute(
                "AllGather",
                mybir.AluOpType.bypass,
                replica_groups=rg,
                ins=[agin.opt()],
                outs=[agout.opt()],
            )
            return agout

        def mk_slabt(ssb, si):
            st = stp.tile([128, KT, SW], F16, name=f"st{si}", tag="st")
            for r in range(2):
                for c in range(KT):
                    psT = tps.tile([128, 128], F16, name=f"pt{si}_{r}_{c}", tag="psT")
                    nc.tensor.transpose(
                        psT[:], ssb[:, r, 128 * c : 128 * (c + 1)], ident[:]
                    )
                    nc.vector.tensor_copy(st[:, c, 128 * r : 128 * (r + 1)], psT[:])
            return st

        # S1: C2 = A'[Sj,:] @ A'
        ss = product(st1, fA, None, 1)
        ag1 = do_ag(ss, 1)  # full C2
        st2 = mk_slabt(ss, 2)

        # S2: C4 = C2[Sj,:] @ C2
        fC2 = fullp.tile([128, KT, G], F16, name="fC2", tag="full")
        ss = product(st2, fC2, ag1, 2)
        ag2 = do_ag(ss, 2)  # full C4
        st4 = mk_slabt(ss, 4)

        # S3: C8 = C4[Sj,:] @ C4
        fC4 = fullp.tile([128, KT, G], F16, name="fC4", tag="full")
        ss = product(st4, fC4, ag2, 3)
        st8 = mk_slabt(ss, 8)

        # S4: C12 = C8[Sj,:] @ C4  (C4 still resident)
        ss = product(st8, fC4, None, 4)
        ag3 = do_ag(ss, 4)  # full C12
        st12 = mk_slabt(ss, 12)

        # S5: C24 = C12[Sj,:] @ C12
        fC12 = fullp.tile([128, KT, G], F16, name="fC12", tag="full")
        ss = product(st12, fC12, ag3, 5)
        ag4 = do_ag(ss, 5)  # full C24
        st24 = mk_slabt(ss, 24)

        # S6: C48 = C24[Sj,:] @ C24
        fC24 = fullp.tile([128, KT, G], F16, name="fC24", tag="full")
        ss = product(st24, fC24, ag4, 6)
        st48 = mk_slabt(ss, 48)

        # S7: C60 = C48[Sj,:] @ C12  (C12 still resident)
        ss = product(st48, fC12, None, 7)
        ag5 = do_ag(ss, 7)  # full C60

        # Apply: y[Bj,:] = x[Bj,:] @ C60
        fC60 = fullp.tile([128, KT, G], F16, name="fC60", tag="full")
        for k in range(KT):
            nc.sync.dma_start(fC60[:, k, :], ag5[128 * k : 128 * (k + 1), :])
        for mr in range(4):
            pts = [
                mmps.tile([128, 512], F32, name=f"py{mr}_{mc}", tag="mm")
                for mc in range(4)
            ]
            for k in range(KT):
                for mc in range(4):
                    nc.tensor.matmul(
                        pts[mc][:],
                        xsb[:, k, 128 * mr : 128 * (mr + 1)],
                        fC60[:, k, 512 * mc : 512 * (mc + 1)],
                        start=(k == 0),
                        stop=(k == KT - 1),
                    )
            for mc in range(4):
                yt = ypool.tile([128, 512], F32, name=f"yt{mr}_{mc}", tag="yt")
                nc.vector.tensor_copy(yt[:], pts[mc][:])
                nc.sync.dma_start(
                    yj[128 * mr : 128 * (mr + 1), 512 * mc : 512 * (mc + 1)], yt[:]
                )
    nc.compile()
    return nc


def _prep(x, W):
    """Host prep: rescale so the fp16 chain stays O(1); fp16 casts."""
    A = np.ascontiguousarray(W.T.astype(np.float64))
    rng = np.random.default_rng(0)
    v = rng.standard_normal(_G)
    growth = []
    for _ in range(60):
        v2 = A @ v
        n2 = np.linalg.norm(v2)
        growth.append(n2 / np.linalg.norm(v))
        v = v2 / n2
    rho = float(np.exp(np.mean(np.log(growth[20:]))))
    c = 1.0 / rho
    a16 = np.ascontiguousarray((c * A).astype(np.float16))
    x16t = np.ascontiguousarray(x.astype(np.float16).T)
    return a16, x16t, c


def kernel(x, W):
    from concourse.bass_utils import run_bass_kernel_spmd

    if "nc" not in _cache:
        _cache["nc"] = _build()
    nc = _cache["nc"]

    x = np.asarray(x, dtype=np.float32)
    W = np.asarray(W, dtype=np.float32)
    a16, x16t, c = _prep(x, W)
    in_maps = [
        {
            "afull": a16,
            "slabt0": np.ascontiguousarray(a16[_SW * j : _SW * (j + 1), :].T),
            "xbj": np.ascontiguousarray(x16t[:, _BW * j : _BW * (j + 1)]),
        }
        for j in range(_NC)
    ]
    res = run_bass_kernel_spmd(nc, in_maps, core_ids=list(range(_NC)))
    _cache["last_exec_time_ns"] = res.exec_time_ns
    _cache["last_results"] = res
    scale = np.float64(c) ** -60
    y = np.concatenate(
        [res.results[j]["yj"].astype(np.float64) * scale for j in range(_NC)], axis=0
    ).astype(np.float32)
    return y


# revision 5
# speedup vs baseline: 1.8099x; 1.0260x over previous
"""Trainium2 kernel for nn_IteratedLinearNet: y = x @ (W.T)^60.

Strategy (8 NeuronCores, single SPMD launch):
  - A' = c*W.T with c = 1/spectral_radius(W.T) estimated host-side, so all
    powers of A' stay O(1) and the whole chain runs in float16 at full PE
    rate with half the DMA/collective bytes of fp32; c^-60 undone on host.
  - row-slab matrix-power chain: core j computes C_{a+b}[Sj,:] =
    C_a[Sj,:] @ C_b with the transposed own slab as stationary operand and
    a replicated full C_b.  Chain 2,3,4,8,12,24,48,60: C3 and C4 multiply
    by the resident input A', so only C4, C12, C24, C60 are gathered -> 4
    AllGathers total.
  - each product is split into column halves; each half is AllGathered as
    soon as it is staged and the consuming product processes the gathered
    halves in column rounds, so communication pipelines with compute.
  - slab transposes for the next stationary operand run on the DMA XBAR
    (dma transpose), keeping TensorE/DVE free and PSUM available for
    8-bank matmul rounds.
  - final apply is batch-data-parallel: y[Bj,:] = x[Bj,:] @ C60.
  - a dummy warmup AllGather absorbs the one-time collective barrier/ncfw
    ramp while the first products (which need no comm) run.

Self-contained: builds/compiles on first call and caches the module.
"""

import numpy as np

_G = 2048
_B = 4096
_NC = 8
_SW = _G // _NC  # 256 slab rows per core
_BW = _B // _NC  # 512 batch rows per core
_KT = _G // 128  # 16 K-chunks
_H = _G // 2  # 1024 column half

_cache = {}


def _build():
    from contextlib import ExitStack

    import concourse.tile as tile
    from concourse import bacc, masks, mybir

    F16 = mybir.dt.float16
    F32 = mybir.dt.float32
    G, KT, SW, BW, H = _G, _KT, _SW, _BW, _H

    nc = bacc.Bacc(None, target_bir_lowering=False, num_devices=_NC)
    afull = nc.declare_dram_parameter("afull", [G, G], F16, isOutput=False)
    slabt0 = nc.declare_dram_parameter("slabt0", [G, SW], F16, isOutput=False)
    xbj = nc.declare_dram_parameter("xbj", [G, BW], F16, isOutput=False)
    yj = nc.declare_dram_parameter("yj", [BW, G], F32, isOutput=True)

    rg = [list(range(_NC))]

    with ExitStack() as ctx:
        tc = ctx.enter_context(tile.TileContext(nc))
        fullp = ctx.enter_context(tc.tile_pool(name="fullp", bufs=2))
        stp = ctx.enter_context(tc.tile_pool(name="stp", bufs=2))
        ssp = ctx.enter_context(tc.tile_pool(name="ssp", bufs=4))
        misc = ctx.enter_context(tc.tile_pool(name="misc", bufs=1))
        ypool = ctx.enter_context(tc.tile_pool(name="ypool", bufs=4))
        mmps = ctx.enter_context(tc.tile_pool(name="mmps", bufs=8, space="PSUM"))
        dram = ctx.enter_context(tc.tile_pool(name="dram", bufs=2, space="DRAM"))

        ident32 = misc.tile([128, 128], F32, name="ident32", tag="ident32")
        masks.make_identity(nc, ident32[:])

        # warmup collective: absorbs the one-time barrier + ncfw ramp while
        # the first products (which need no comm) run.
        wtile = misc.tile([128, 16], F16, name="warm", tag="warm")
        nc.vector.tensor_copy(wtile[:], ident32[:, 0:16])
        win = dram.tile([128, 16], F16, name="warmin", tag="warmin")
        nc.sync.dma_start(win[:], wtile[:])
        wout = dram.tile(
            [128 * _NC, 16], F16, name="warmout", tag="warmout", addr_space="Shared"
        )
        nc.gpsimd.collective_compute(
            "AllGather",
            mybir.AluOpType.bypass,
            replica_groups=rg,
            ins=[win.opt()],
            outs=[wout.opt()],
        )

        # step-1 stationary slab + streamed A' (left half first) + x^T slice
        st1 = stp.tile([128, KT, SW], F16, name="st1", tag="st")
        fA = fullp.tile([128, KT, G], F16, name="fA", tag="full")
        for k in range(KT):
            nc.sync.dma_start(st1[:, k, :], slabt0[128 * k : 128 * (k + 1), :])
            nc.sync.dma_start(
                fA[:, k, 0:H], afull[128 * k : 128 * (k + 1), 0:H]
            )
        for k in range(KT):
            nc.sync.dma_start(
                fA[:, k, H:G], afull[128 * k : 128 * (k + 1), H:G]
            )
        xsb = misc.tile([128, KT, BW], F16, name="xsb", tag="xsb")
        for k in range(KT):
            nc.sync.dma_start(xsb[:, k, :], xbj[128 * k : 128 * (k + 1), :])

        def product(st, F, si, reload_srcs=None, do_ag=False, make_st=True):
            """Emit one slab product C_next[Sj,:] = slab(st) @ F.

            reload_srcs: optional (agout_h0, agout_h1) to stream F from.
            do_ag: AllGather the output slab (by column halves); returns
                   (st_next, (agout_h0, agout_h1)).
            make_st: build the transposed slab for the next product via
                   DMA XBAR transposes.
            """
            if reload_srcs is not None:
                for h in range(2):
                    for k in range(KT):
                        nc.sync.dma_start(
                            F[:, k, H * h : H * (h + 1)],
                            reload_srcs[h][128 * k : 128 * (k + 1), :],
                        )
            st_next = (
                stp.tile([128, KT, SW], F16, name=f"st_{si}", tag="st")
                if make_st
                else None
            )
            ag_outs = []
            for rnd in range(2):
                pts = [
                    [
                        mmps.tile(
                            [128, 512], F32, name=f"p{si}_{rnd}_{mr}_{mci}", tag="mm"
                        )
                        for mci in range(2)
                    ]
                    for mr in range(2)
                ]
                for k in range(KT):
                    for mr in range(2):
                        for mci in range(2):
                            mc = 2 * rnd + mci
                            nc.tensor.matmul(
                                pts[mr][mci][:],
                                st[:, k, 128 * mr : 128 * (mr + 1)],
                                F[:, k, 512 * mc : 512 * (mc + 1)],
                                start=(k == 0),
                                stop=(k == KT - 1),
                            )
                ssb = ssp.tile([128, 2, H], F16, name=f"ss{si}_{rnd}", tag="ss")
                for mr in range(2):
                    for mci in range(2):
                        nc.vector.tensor_copy(
                            ssb[:, mr, 512 * mci : 512 * (mci + 1)], pts[mr][mci][:]
                        )
                if do_ag:
                    agin = dram.tile([SW, H], F16, name=f"agin{si}_{rnd}", tag="agin")
                    for r in range(2):
                        nc.scalar.dma_start(agin[128 * r : 128 * (r + 1), :], ssb[:, r, :])
                    agout = dram.tile(
                        [G, H],
                        F16,
                        name=f"agout{si}_{rnd}",
                        tag="agout",
                        addr_space="Shared",
                    )
                    nc.gpsimd.collective_compute(
                        "AllGather",
                        mybir.AluOpType.bypass,
                        replica_groups=rg,
                        ins=[agin.opt()],
                        outs=[agout.opt()],
                    )
                    ag_outs.append(agout)
                if make_st:
                    # slabT rows 1024*rnd..+1023 = transpose of this half
                    for r in range(2):
                        nc.scalar.dma_start(
                            st_next[:, 8 * rnd : 8 * (rnd + 1), 128 * r : 128 * (r + 1)],
                            ssb[:, r, :],
                            transpose=True,
                        )
            return st_next, (tuple(ag_outs) if do_ag else None)

        # chain: 2, 3, 4 (vs resident A'), 8, 12 (vs C4), 24 (vs C12),
        #        48 (vs C24), 60 (vs resident C12)
        st2, _ = product(st1, fA, "c2")
        st3, _ = product(st2, fA, "c3")
        st4, ag4 = product(st3, fA, "c4", do_ag=True)

        fC4 = fullp.tile([128, KT, G], F16, name="fC4", tag="full")
        st8, _ = product(st4, fC4, "c8", reload_srcs=ag4)
        st12, ag12 = product(st8, fC4, "c12", do_ag=True)

        fC12 = fullp.tile([128, KT, G], F16, name="fC12", tag="full")
        st24, ag24 = product(st12, fC12, "c24", reload_srcs=ag12, do_ag=True)

        fC24 = fullp.tile([128, KT, G], F16, name="fC24", tag="full")
        st48, _ = product(st24, fC24, "c48", reload_srcs=ag24)
        _, ag60 = product(st48, fC12, "c60", do_ag=True, make_st=False)

        # apply: y[Bj,:] = x[Bj,:] @ C60, column halves follow the C60 AG
        fC60 = fullp.tile([128, KT, G], F16, name="fC60", tag="full")
        for h in range(2):
            for k in range(KT):
                nc.sync.dma_start(
                    fC60[:, k, H * h : H * (h + 1)],
                    ag60[h][128 * k : 128 * (k + 1), :],
                )
        for rnd in range(2):
            pts = [
                [
                    mmps.tile([128, 512], F32, name=f"py{rnd}_{mr}_{mci}", tag="mm")
                    for mci in range(2)
                ]
                for mr in range(4)
            ]
            for k in range(KT):
                for mr in range(4):
                    for mci in range(2):
                        mc = 2 * rnd + mci
                        nc.tensor.matmul(
                            pts[mr][mci][:],
                            xsb[:, k, 128 * mr : 128 * (mr + 1)],
                            fC60[:, k, 512 * mc : 512 * (mc + 1)],
                            start=(k == 0),
                            stop=(k == KT - 1),
                        )
            for mr in range(4):
                for mci in range(2):
                    mc = 2 * rnd + mci
                    yt = ypool.tile([128, 512], F32, name=f"yt{rnd}_{mr}_{mci}", tag="yt")
                    nc.vector.tensor_copy(yt[:], pts[mr][mci][:])
                    nc.scalar.dma_start(
                        yj[128 * mr : 128 * (mr + 1), 512 * mc : 512 * (mc + 1)],
                        yt[:],
                    )
    nc.compile()
    return nc


def _prep(x, W):
    """Host prep: rescale so the fp16 chain stays O(1); fp16 casts."""
    A = np.ascontiguousarray(W.T.astype(np.float64))
    rng = np.random.default_rng(0)
    v = rng.standard_normal(_G)
    growth = []
    for _ in range(60):
        v2 = A @ v
        n2 = np.linalg.norm(v2)
        growth.append(n2 / np.linalg.norm(v))
        v = v2 / n2
    rho = float(np.exp(np.mean(np.log(growth[20:]))))
    c = 1.0 / rho
    a16 = np.ascontiguousarray((c * A).astype(np.float16))
    x16t = np.ascontiguousarray(x.astype(np.float16).T)
    return a16, x16t, c


def kernel(x, W):
    from concourse.bass_utils import run_bass_kernel_spmd

    if "nc" not in _cache:
        _cache["nc"] = _build()
    nc = _cache["nc"]

    x = np.asarray(x, dtype=np.float32)
    W = np.asarray(W, dtype=np.float32)
    a16, x16t, c = _prep(x, W)
    in_maps = [
        {
            "afull": a16,
            "slabt0": np.ascontiguousarray(a16[_SW * j : _SW * (j + 1), :].T),
            "xbj": np.ascontiguousarray(x16t[:, _BW * j : _BW * (j + 1)]),
        }
        for j in range(_NC)
    ]
    res = run_bass_kernel_spmd(nc, in_maps, core_ids=list(range(_NC)))
    _cache["last_exec_time_ns"] = res.exec_time_ns
    _cache["last_results"] = res
    scale = np.float64(c) ** -60
    y = np.concatenate(
        [res.results[j]["yj"].astype(np.float64) * scale for j in range(_NC)], axis=0
    ).astype(np.float32)
    return y


# revision 6
# speedup vs baseline: 1.8787x; 1.0380x over previous
"""Trainium2 kernel for nn_IteratedLinearNet: y = x @ (W.T)^60.

Strategy (8 NeuronCores, single SPMD launch):
  - A' = c*W.T with c = 1/spectral_radius(W.T) estimated host-side, so all
    powers of A' stay O(1) and the whole chain runs in float16 at full PE
    rate with half the DMA/collective bytes of fp32; c^-60 undone on host.
  - row-slab matrix-power chain: core j computes C_{a+b}[Sj,:] =
    C_a[Sj,:] @ C_b with the transposed own slab as stationary operand and
    a replicated full C_b.  Chain 2,3,4,8,12,24,36,48,60: products 3,4 use
    the resident input A' and 36,48,60 reuse the resident C12, so only
    C4, C12, C60 are gathered -> 3 AllGathers total.
  - gathered matrices stream from DRAM K-chunk by K-chunk on BOTH hwdge
    queues (alternating), so reloads never pace the matmuls.
  - the stationary slab transpose for the next product runs on TensorE but
    is interleaved chunk-by-chunk just ahead of the consuming matmuls, so
    it pipelines instead of serializing.
  - keep-warm junk matmuls run during AllGather windows so the PE clock
    (HAM gate) stays at high p-state.
  - final apply is batch-data-parallel: y[Bj,:] = x[Bj,:] @ C60.
  - a dummy warmup AllGather absorbs the one-time collective barrier/ncfw
    ramp while the first products (which need no comm) run.

Self-contained: builds/compiles on first call and caches the module.
"""

import numpy as np

_G = 2048
_B = 4096
_NC = 8
_SW = _G // _NC  # 256 slab rows per core
_BW = _B // _NC  # 512 batch rows per core
_KT = _G // 128  # 16 K-chunks
_H = _G // 2

_cache = {}


def _build():
    from contextlib import ExitStack

    import concourse.tile as tile
    from concourse import bacc, masks, mybir

    F16 = mybir.dt.float16
    F32 = mybir.dt.float32
    G, KT, SW, BW = _G, _KT, _SW, _BW

    nc = bacc.Bacc(None, target_bir_lowering=False, num_devices=_NC)
    afull = nc.declare_dram_parameter("afull", [G, G], F16, isOutput=False)
    slabt0 = nc.declare_dram_parameter("slabt0", [G, SW], F16, isOutput=False)
    xbj = nc.declare_dram_parameter("xbj", [G, BW], F16, isOutput=False)
    yj = nc.declare_dram_parameter("yj", [BW, G], F32, isOutput=True)

    rg = [list(range(_NC))]
    qeng = [nc.sync, nc.scalar]  # two hwdge DMA queues

    with ExitStack() as ctx:
        tc = ctx.enter_context(tile.TileContext(nc))
        fullp = ctx.enter_context(tc.tile_pool(name="fullp", bufs=2))
        stp = ctx.enter_context(tc.tile_pool(name="stp", bufs=2))
        ssp = ctx.enter_context(tc.tile_pool(name="ssp", bufs=4))
        misc = ctx.enter_context(tc.tile_pool(name="misc", bufs=1))
        ypool = ctx.enter_context(tc.tile_pool(name="ypool", bufs=4))
        mmps = ctx.enter_context(tc.tile_pool(name="mmps", bufs=6, space="PSUM"))
        tps = ctx.enter_context(tc.tile_pool(name="tps", bufs=2, space="PSUM"))
        dram = ctx.enter_context(tc.tile_pool(name="dram", bufs=2, space="DRAM"))

        ident32 = misc.tile([128, 128], F32, name="ident32", tag="ident32")
        masks.make_identity(nc, ident32[:])
        ident = misc.tile([128, 128], F16, name="ident", tag="ident")
        nc.vector.tensor_copy(ident[:], ident32[:])

        # warmup collective: absorbs the one-time barrier + ncfw ramp while
        # the first products (which need no comm) run.
        wtile = misc.tile([128, 16], F16, name="warm", tag="warm")
        nc.vector.tensor_copy(wtile[:], ident32[:, 0:16])
        win = dram.tile([128, 16], F16, name="warmin", tag="warmin")
        nc.scalar.dma_start(win[:], wtile[:])
        wout = dram.tile(
            [128 * _NC, 16], F16, name="warmout", tag="warmout", addr_space="Shared"
        )
        nc.gpsimd.collective_compute(
            "AllGather",
            mybir.AluOpType.bypass,
            replica_groups=rg,
            ins=[win.opt()],
            outs=[wout.opt()],
        )

        # step-1 stationary slab + streamed A' on both queues
        st1 = stp.tile([128, KT, SW], F16, name="st1", tag="st")
        fA = fullp.tile([128, KT, G], F16, name="fA", tag="full")
        for k in range(KT):
            nc.sync.dma_start(st1[:, k, :], slabt0[128 * k : 128 * (k + 1), :])
            qeng[k % 2].dma_start(fA[:, k, :], afull[128 * k : 128 * (k + 1), :])
        xsb = misc.tile([128, KT, BW], F16, name="xsb", tag="xsb")
        for k in range(KT):
            qeng[k % 2].dma_start(xsb[:, k, :], xbj[128 * k : 128 * (k + 1), :])

        def product(st, prev_ssbs, F, si, reload_src=None, do_ag=False, make_st=True):
            """Emit one slab product C_next[Sj,:] = slab(st) @ F.

            st: this product's stationary tile; if prev_ssbs is given it is
                built chunk-by-chunk via TensorE transposes interleaved
                ahead of the consuming matmuls.
            reload_src: optional DRAM source (AG output) to stream F from.
            Returns (ssbs, st_next, agout).
            """
            if reload_src is not None:
                for k in range(KT):
                    qeng[k % 2].dma_start(
                        F[:, k, :], reload_src[128 * k : 128 * (k + 1), :]
                    )
            ssbs = []
            for rnd in range(2):
                pts = [
                    [
                        mmps.tile(
                            [128, 512], F32, name=f"p{si}_{rnd}_{mr}_{mci}", tag="mm"
                        )
                        for mci in range(2)
                    ]
                    for mr in range(2)
                ]
                for k in range(KT):
                    if rnd == 0 and prev_ssbs is not None:
                        src = prev_ssbs[k // 8]
                        off = 128 * (k % 8)
                        for r in range(2):
                            psT = tps.tile(
                                [128, 128], F16, name=f"pt{si}_{k}_{r}", tag="psT"
                            )
                            nc.tensor.transpose(
                                psT[:], src[:, r, off : off + 128], ident[:]
                            )
                            nc.vector.tensor_copy(
                                st[:, k, 128 * r : 128 * (r + 1)], psT[:]
                            )
                    for mr in range(2):
                        for mci in range(2):
                            mc = 2 * rnd + mci
                            nc.tensor.matmul(
                                pts[mr][mci][:],
                                st[:, k, 128 * mr : 128 * (mr + 1)],
                                F[:, k, 512 * mc : 512 * (mc + 1)],
                                start=(k == 0),
                                stop=(k == KT - 1),
                            )
                ssb = ssp.tile([128, 2, _H], F16, name=f"ss{si}_{rnd}", tag="ss")
                for mr in range(2):
                    for mci in range(2):
                        nc.vector.tensor_copy(
                            ssb[:, mr, 512 * mci : 512 * (mci + 1)], pts[mr][mci][:]
                        )
                ssbs.append(ssb)
            agout = None
            if do_ag:
                agin = dram.tile([SW, G], F16, name=f"agin{si}", tag="agin")
                for rnd in range(2):
                    for r in range(2):
                        nc.scalar.dma_start(
                            agin[128 * r : 128 * (r + 1), _H * rnd : _H * (rnd + 1)],
                            ssbs[rnd][:, r, :],
                        )
                agout = dram.tile(
                    [G, G], F16, name=f"agout{si}", tag="agout", addr_space="Shared"
                )
                nc.gpsimd.collective_compute(
                    "AllGather",
                    mybir.AluOpType.bypass,
                    replica_groups=rg,
                    ins=[agin.opt()],
                    outs=[agout.opt()],
                )
                # keep-warm junk during the AG window (results unused)
                for jj in range(32):
                    jt = mmps.tile([128, 512], F32, name=f"junk{si}_{jj}", tag="mm")
                    nc.tensor.matmul(
                        jt[:],
                        xsb[:, 0, 0:128],
                        xsb[:, 0, :],
                        start=True,
                        stop=True,
                    )
            st_next = (
                stp.tile([128, KT, SW], F16, name=f"stn_{si}", tag="st")
                if make_st
                else None
            )
            return ssbs, st_next, agout

        # chain: 2,3,4 (rhs = resident A'), 8,12 (rhs = C4), 24 (rhs = C12),
        #        36,48,60 (rhs = resident C12)
        ss, st2, _ = product(st1, None, fA, "c2")
        ss, st3, _ = product(st2, ss, fA, "c3")
        ss, st4, ag4 = product(st3, ss, fA, "c4", do_ag=True)

        fC4 = fullp.tile([128, KT, G], F16, name="fC4", tag="full")
        ss, st8, _ = product(st4, ss, fC4, "c8", reload_src=ag4)
        ss, st12, ag12 = product(st8, ss, fC4, "c12", do_ag=True)

        fC12 = fullp.tile([128, KT, G], F16, name="fC12", tag="full")
        ss, st24, _ = product(st12, ss, fC12, "c24", reload_src=ag12)
        ss, st36, _ = product(st24, ss, fC12, "c36")
        ss, st48, _ = product(st36, ss, fC12, "c48")
        ss, _, ag60 = product(st48, ss, fC12, "c60", do_ag=True, make_st=False)

        # apply: y[Bj,:] = x[Bj,:] @ C60
        fC60 = fullp.tile([128, KT, G], F16, name="fC60", tag="full")
        for k in range(KT):
            qeng[k % 2].dma_start(fC60[:, k, :], ag60[128 * k : 128 * (k + 1), :])
        for mr in range(4):
            pts = [
                mmps.tile([128, 512], F32, name=f"py{mr}_{mc}", tag="mm")
                for mc in range(4)
            ]
            for k in range(KT):
                for mc in range(4):
                    nc.tensor.matmul(
                        pts[mc][:],
                        xsb[:, k, 128 * mr : 128 * (mr + 1)],
                        fC60[:, k, 512 * mc : 512 * (mc + 1)],
                        start=(k == 0),
                        stop=(k == KT - 1),
                    )
            for mc in range(4):
                yt = ypool.tile([128, 512], F32, name=f"yt{mr}_{mc}", tag="yt")
                nc.vector.tensor_copy(yt[:], pts[mc][:])
                qeng[mc % 2].dma_start(
                    yj[128 * mr : 128 * (mr + 1), 512 * mc : 512 * (mc + 1)], yt[:]
                )
    nc.compile()
    return nc


def _prep(x, W):
    """Host prep: rescale so the fp16 chain stays O(1); fp16 casts."""
    A = np.ascontiguousarray(W.T.astype(np.float64))
    rng = np.random.default_rng(0)
    v = rng.standard_normal(_G)
    growth = []
    for _ in range(60):
        v2 = A @ v
        n2 = np.linalg.norm(v2)
        growth.append(n2 / np.linalg.norm(v))
        v = v2 / n2
    rho = float(np.exp(np.mean(np.log(growth[20:]))))
    c = 1.0 / rho
    a16 = np.ascontiguousarray((c * A).astype(np.float16))
    x16t = np.ascontiguousarray(x.astype(np.float16).T)
    return a16, x16t, c


def kernel(x, W):
    from concourse.bass_utils import run_bass_kernel_spmd

    if "nc" not in _cache:
        _cache["nc"] = _build()
    nc = _cache["nc"]

    x = np.asarray(x, dtype=np.float32)
    W = np.asarray(W, dtype=np.float32)
    a16, x16t, c = _prep(x, W)
    in_maps = [
        {
            "afull": a16,
            "slabt0": np.ascontiguousarray(a16[_SW * j : _SW * (j + 1), :].T),
            "xbj": np.ascontiguousarray(x16t[:, _BW * j : _BW * (j + 1)]),
        }
        for j in range(_NC)
    ]
    res = run_bass_kernel_spmd(nc, in_maps, core_ids=list(range(_NC)))
    _cache["last_exec_time_ns"] = res.exec_time_ns
    _cache["last_results"] = res
    scale = np.float64(c) ** -60
    y = np.concatenate(
        [res.results[j]["yj"].astype(np.float64) * scale for j in range(_NC)], axis=0
    ).astype(np.float32)
    return y


# revision 8
# speedup vs baseline: 1.9412x; 1.0333x over previous
"""Trainium2 kernel for nn_IteratedLinearNet: y = x @ (W.T)^60.

Strategy (8 NeuronCores, single SPMD launch):
  - A' = c*W.T with c = 1/spectral_radius(W.T) estimated host-side, so all
    powers of A' stay O(1) and the whole chain runs in float16 at full PE
    rate with half the DMA/collective bytes of fp32; c^-60 undone on host.
  - row-slab matrix-power chain: core j computes C_{a+b}[Sj,:] =
    C_a[Sj,:] @ C_b with the transposed own slab as stationary operand and
    a replicated full C_b.  Chain 2,3,4,8,12,24,36,48: products 3,4 use
    the resident input A' and 36,48 reuse the resident C12, so only C4,
    C12, C48 are gathered -> 3 AllGathers total.
  - the apply is split as y = (x @ C12) @ C48: u^T = C12^T @ x^T runs
    during the C48 AllGather (computed directly in transposed form with
    the resident C12 as stationary), and filler products C5 = C4 @ A',
    C13 = C12 @ A' occupy the other two AllGather windows, so TensorE
    never idles long enough for the HAM clock gate to throttle.
  - gathered matrices stream from DRAM K-chunk by K-chunk on BOTH hwdge
    queues (alternating), so reloads never pace the matmuls.
  - stationary-slab transposes run on TensorE, interleaved chunk-by-chunk
    just ahead of the consuming matmuls.
  - a dummy warmup AllGather absorbs the one-time collective barrier/ncfw
    ramp while the first products (which need no comm) run.

Self-contained: builds/compiles on first call and caches the module.
"""

import numpy as np

_G = 2048
_B = 4096
_NC = 8
_SW = _G // _NC  # 256 slab rows per core
_BW = _B // _NC  # 512 batch rows per core
_KT = _G // 128  # 16 K-chunks
_H = _G // 2

_cache = {}


def _build():
    from contextlib import ExitStack

    import concourse.tile as tile
    from concourse import bacc, masks, mybir

    F16 = mybir.dt.float16
    F32 = mybir.dt.float32
    G, KT, SW, BW = _G, _KT, _SW, _BW

    nc = bacc.Bacc(None, target_bir_lowering=False, num_devices=_NC)
    afull = nc.declare_dram_parameter("afull", [G, G], F16, isOutput=False)
    slabt0 = nc.declare_dram_parameter("slabt0", [G, SW], F16, isOutput=False)
    xbj = nc.declare_dram_parameter("xbj", [G, BW], F16, isOutput=False)
    yj = nc.declare_dram_parameter("yj", [BW, G], F32, isOutput=True)

    rg = [list(range(_NC))]

    with ExitStack() as ctx:
        tc = ctx.enter_context(tile.TileContext(nc))
        fullp = ctx.enter_context(tc.tile_pool(name="fullp", bufs=2))
        stp = ctx.enter_context(tc.tile_pool(name="stp", bufs=2))
        ssp = ctx.enter_context(tc.tile_pool(name="ssp", bufs=4))
        misc = ctx.enter_context(tc.tile_pool(name="misc", bufs=1))
        ypool = ctx.enter_context(tc.tile_pool(name="ypool", bufs=4))
        mmps = ctx.enter_context(tc.tile_pool(name="mmps", bufs=6, space="PSUM"))
        tps = ctx.enter_context(tc.tile_pool(name="tps", bufs=2, space="PSUM"))
        dram = ctx.enter_context(tc.tile_pool(name="dram", bufs=2, space="DRAM"))

        qeng = [nc.sync, nc.scalar]  # two hwdge DMA queues

        ident32 = misc.tile([128, 128], F32, name="ident32", tag="ident32")
        masks.make_identity(nc, ident32[:])
        ident = misc.tile([128, 128], F16, name="ident", tag="ident")
        nc.vector.tensor_copy(ident[:], ident32[:])

        # warmup collective: absorbs the one-time barrier + ncfw ramp while
        # the first products (which need no comm) run.
        wtile = misc.tile([128, 16], F16, name="warm", tag="warm")
        nc.vector.tensor_copy(wtile[:], ident32[:, 0:16])
        win = dram.tile([128, 16], F16, name="warmin", tag="warmin")
        nc.scalar.dma_start(win[:], wtile[:])
        wout = dram.tile(
            [128 * _NC, 16], F16, name="warmout", tag="warmout", addr_space="Shared"
        )
        nc.gpsimd.collective_compute(
            "AllGather",
            mybir.AluOpType.bypass,
            replica_groups=rg,
            ins=[win.opt()],
            outs=[wout.opt()],
        )

        # step-1 stationary slab + streamed A' on both queues + x^T slice
        st1 = stp.tile([128, KT, SW], F16, name="st1", tag="st")
        fA = fullp.tile([128, KT, G], F16, name="fA", tag="full")
        for k in range(KT):
            nc.sync.dma_start(st1[:, k, :], slabt0[128 * k : 128 * (k + 1), :])
            qeng[k % 2].dma_start(fA[:, k, :], afull[128 * k : 128 * (k + 1), :])
        xsb = misc.tile([128, KT, BW], F16, name="xsb", tag="xsb")
        for k in range(KT):
            qeng[k % 2].dma_start(xsb[:, k, :], xbj[128 * k : 128 * (k + 1), :])

        def product(st, prev_ssbs, F, si, reload_src=None, do_ag=False):
            """Emit one slab product = slab(st) @ F.

            If prev_ssbs is given, st is built chunk-by-chunk from them via
            TensorE transposes interleaved ahead of the consuming matmuls.
            """
            if reload_src is not None:
                for k in range(KT):
                    qeng[k % 2].dma_start(
                        F[:, k, :], reload_src[128 * k : 128 * (k + 1), :]
                    )
            ssbs = []
            for rnd in range(2):
                pts = [
                    [
                        mmps.tile(
                            [128, 512], F32, name=f"p{si}_{rnd}_{mr}_{mci}", tag="mm"
                        )
                        for mci in range(2)
                    ]
                    for mr in range(2)
                ]
                for k in range(KT):
                    if rnd == 0 and prev_ssbs is not None:
                        src = prev_ssbs[k // 8]
                        off = 128 * (k % 8)
                        for r in range(2):
                            psT = tps.tile(
                                [128, 128], F16, name=f"pt{si}_{k}_{r}", tag="psT"
                            )
                            nc.tensor.transpose(
                                psT[:], src[:, r, off : off + 128], ident[:]
                            )
                            nc.vector.tensor_copy(
                                st[:, k, 128 * r : 128 * (r + 1)], psT[:]
                            )
                    for mr in range(2):
                        for mci in range(2):
                            mc = 2 * rnd + mci
                            nc.tensor.matmul(
                                pts[mr][mci][:],
                                st[:, k, 128 * mr : 128 * (mr + 1)],
                                F[:, k, 512 * mc : 512 * (mc + 1)],
                                start=(k == 0),
                                stop=(k == KT - 1),
                            )
                ssb = ssp.tile([128, 2, _H], F16, name=f"ss{si}_{rnd}", tag="ss")
                for mr in range(2):
                    for mci in range(2):
                        nc.vector.tensor_copy(
                            ssb[:, mr, 512 * mci : 512 * (mci + 1)], pts[mr][mci][:]
                        )
                ssbs.append(ssb)
            agout = None
            if do_ag:
                agin = dram.tile([SW, G], F16, name=f"agin{si}", tag="agin")
                for rnd in range(2):
                    for r in range(2):
                        nc.scalar.dma_start(
                            agin[128 * r : 128 * (r + 1), _H * rnd : _H * (rnd + 1)],
                            ssbs[rnd][:, r, :],
                        )
                agout = dram.tile(
                    [G, G], F16, name=f"agout{si}", tag="agout", addr_space="Shared"
                )
                nc.gpsimd.collective_compute(
                    "AllGather",
                    mybir.AluOpType.bypass,
                    replica_groups=rg,
                    ins=[agin.opt()],
                    outs=[agout.opt()],
                )
            return ssbs, agout

        def new_st(si):
            return stp.tile([128, KT, SW], F16, name=f"st_{si}", tag="st")

        # chain
        ss, _ = product(st1, None, fA, "c2")
        st2 = new_st("c3")
        ss, _ = product(st2, ss, fA, "c3")
        st3 = new_st("c4")
        ss, ag4 = product(st3, ss, fA, "c4", do_ag=True)

        # filler C5 = C4 @ A' occupies the AG(C4) window and builds st4
        st4 = new_st("c8")
        ss, _ = product(st4, ss, fA, "f5")

        fC4 = fullp.tile([128, KT, G], F16, name="fC4", tag="full")
        ss, _ = product(st4, None, fC4, "c8", reload_src=ag4)
        st8 = new_st("c12")
        ss, ag12 = product(st8, ss, fC4, "c12", do_ag=True)

        # filler C13 = C12 @ A' occupies the AG(C12) window and builds st12
        st12 = new_st("c24")
        ss, _ = product(st12, ss, fA, "f13")

        fC12 = fullp.tile([128, KT, G], F16, name="fC12", tag="full")
        ss, _ = product(st12, None, fC12, "c24", reload_src=ag12)
        st24 = new_st("c36")
        ss, _ = product(st24, ss, fC12, "c36")
        st36 = new_st("c48")
        ss, ag48 = product(st36, ss, fC12, "c48", do_ag=True)

        # u^T = C12^T @ x^T during the AG(C48) window (C12 stationary)
        ut = misc.tile([128, KT, BW], F16, name="ut", tag="ut")
        for m in range(KT):
            pu = mmps.tile([128, BW], F32, name=f"pu{m}", tag="mm")
            for k in range(KT):
                nc.tensor.matmul(
                    pu[:],
                    fC12[:, k, 128 * m : 128 * (m + 1)],
                    xsb[:, k, :],
                    start=(k == 0),
                    stop=(k == KT - 1),
                )
            nc.vector.tensor_copy(ut[:, m, :], pu[:])

        # y[Bj,:] = u[Bj,:] @ C48
        fC48 = fullp.tile([128, KT, G], F16, name="fC48", tag="full")
        for k in range(KT):
            qeng[k % 2].dma_start(fC48[:, k, :], ag48[128 * k : 128 * (k + 1), :])
        for mr in range(4):
            pts = [
                mmps.tile([128, 512], F32, name=f"py{mr}_{mc}", tag="mm")
                for mc in range(4)
            ]
            for k in range(KT):
                for mc in range(4):
                    nc.tensor.matmul(
                        pts[mc][:],
                        ut[:, k, 128 * mr : 128 * (mr + 1)],
                        fC48[:, k, 512 * mc : 512 * (mc + 1)],
                        start=(k == 0),
                        stop=(k == KT - 1),
                    )
            for mc in range(4):
                yt = ypool.tile([128, 512], F32, name=f"yt{mr}_{mc}", tag="yt")
                nc.vector.tensor_copy(yt[:], pts[mc][:])
                qeng[mc % 2].dma_start(
                    yj[128 * mr : 128 * (mr + 1), 512 * mc : 512 * (mc + 1)], yt[:]
                )
    nc.compile()
    return nc


def _prep(x, W):
    """Host prep: rescale so the fp16 chain stays O(1); fp16 casts."""
    A = np.ascontiguousarray(W.T.astype(np.float64))
    rng = np.random.default_rng(0)
    v = rng.standard_normal(_G)
    growth = []
    for _ in range(60):
        v2 = A @ v
        n2 = np.linalg.norm(v2)
        growth.append(n2 / np.linalg.norm(v))
        v = v2 / n2
    rho = float(np.exp(np.mean(np.log(growth[20:]))))
    c = 1.0 / rho
    a16 = np.ascontiguousarray((c * A).astype(np.float16))
    x16t = np.ascontiguousarray(x.astype(np.float16).T)
    return a16, x16t, c


def kernel(x, W):
    from concourse.bass_utils import run_bass_kernel_spmd

    if "nc" not in _cache:
        _cache["nc"] = _build()
    nc = _cache["nc"]

    x = np.asarray(x, dtype=np.float32)
    W = np.asarray(W, dtype=np.float32)
    a16, x16t, c = _prep(x, W)
    in_maps = [
        {
            "afull": a16,
            "slabt0": np.ascontiguousarray(a16[_SW * j : _SW * (j + 1), :].T),
            "xbj": np.ascontiguousarray(x16t[:, _BW * j : _BW * (j + 1)]),
        }
        for j in range(_NC)
    ]
    res = run_bass_kernel_spmd(nc, in_maps, core_ids=list(range(_NC)))
    _cache["last_exec_time_ns"] = res.exec_time_ns
    _cache["last_results"] = res
    scale = np.float64(c) ** -60
    y = np.concatenate(
        [res.results[j]["yj"].astype(np.float64) * scale for j in range(_NC)], axis=0
    ).astype(np.float32)
    return y


# revision 11
# speedup vs baseline: 1.9944x; 1.0274x over previous
"""Trainium2 kernel for nn_IteratedLinearNet: y = x @ (W.T)^60.

Strategy (8 NeuronCores, single SPMD launch):
  - A' = c*W.T with c = 1/spectral_radius(W.T) estimated host-side, so all
    powers of A' stay O(1) and the whole chain runs in float16 at full PE
    rate with half the DMA/collective bytes of fp32; c^-60 undone on host.
  - row-slab matrix-power chain: core j computes C_{a+b}[Sj,:] =
    C_a[Sj,:] @ C_b with the transposed own slab as stationary operand and
    a replicated full C_b.  Chain 2,3,4,8,12,24,36,48: products 3,4 use
    the resident input A' and 36,48 reuse the resident C12, so only C4,
    C12, C48 are gathered -> 3 AllGathers total.
  - the apply is split as y = (x @ C12) @ C48: u^T = C12^T @ x^T runs
    during the C48 AllGather (computed directly in transposed form with
    the resident C12 as stationary), and filler products C5 = C4 @ A',
    C13 = C12 @ A' occupy the other two AllGather windows, so TensorE
    never idles long enough for the HAM clock gate to throttle.
  - gathered matrices stream from DRAM K-chunk by K-chunk on BOTH hwdge
    queues (alternating), so reloads never pace the matmuls.
  - stationary-slab transposes run on TensorE, interleaved chunk-by-chunk
    just ahead of the consuming matmuls.
  - a dummy warmup AllGather absorbs the one-time collective barrier/ncfw
    ramp while the first products (which need no comm) run.

Self-contained: builds/compiles on first call and caches the module.
"""

import numpy as np

_G = 2048
_B = 4096
_NC = 8
_SW = _G // _NC  # 256 slab rows per core
_BW = _B // _NC  # 512 batch rows per core
_KT = _G // 128  # 16 K-chunks
_H = _G // 2

_cache = {}


def _build():
    from contextlib import ExitStack

    import concourse.tile as tile
    from concourse import bacc, masks, mybir

    F16 = mybir.dt.float16
    F32 = mybir.dt.float32
    G, KT, SW, BW = _G, _KT, _SW, _BW

    nc = bacc.Bacc(None, target_bir_lowering=False, num_devices=_NC)
    afull = nc.declare_dram_parameter("afull", [G, G], F16, isOutput=False)
    slabt0 = nc.declare_dram_parameter("slabt0", [G, SW], F16, isOutput=False)
    xbj = nc.declare_dram_parameter("xbj", [G, BW], F16, isOutput=False)
    yj = nc.declare_dram_parameter("yj", [BW, G], F32, isOutput=True)

    rg = [list(range(_NC))]

    with ExitStack() as ctx:
        tc = ctx.enter_context(tile.TileContext(nc))
        fullp = ctx.enter_context(tc.tile_pool(name="fullp", bufs=2))
        stp = ctx.enter_context(tc.tile_pool(name="stp", bufs=2))
        ssp = ctx.enter_context(tc.tile_pool(name="ssp", bufs=4))
        misc = ctx.enter_context(tc.tile_pool(name="misc", bufs=1))
        ypool = ctx.enter_context(tc.tile_pool(name="ypool", bufs=4))
        mmps = ctx.enter_context(tc.tile_pool(name="mmps", bufs=6, space="PSUM"))
        tps = ctx.enter_context(tc.tile_pool(name="tps", bufs=2, space="PSUM"))
        dram = ctx.enter_context(tc.tile_pool(name="dram", bufs=2, space="DRAM"))

        qeng = [nc.sync, nc.scalar]  # two hwdge DMA queues

        ident32 = misc.tile([128, 128], F32, name="ident32", tag="ident32")
        masks.make_identity(nc, ident32[:])
        ident = misc.tile([128, 128], F16, name="ident", tag="ident")
        nc.vector.tensor_copy(ident[:], ident32[:])

        # warmup collective: absorbs the one-time barrier + ncfw ramp while
        # the first products (which need no comm) run.
        wtile = misc.tile([128, 16], F16, name="warm", tag="warm")
        nc.vector.tensor_copy(wtile[:], ident32[:, 0:16])
        win = dram.tile([128, 16], F16, name="warmin", tag="warmin")
        nc.scalar.dma_start(win[:], wtile[:])
        wout = dram.tile(
            [128 * _NC, 16], F16, name="warmout", tag="warmout", addr_space="Shared"
        )
        nc.gpsimd.collective_compute(
            "AllGather",
            mybir.AluOpType.bypass,
            replica_groups=rg,
            ins=[win.opt()],
            outs=[wout.opt()],
        )

        # pre-warm the PE clock before the first real product
        for jj in range(64):
            jt = tps.tile([128, 128], F16, name=f"jw{jj}", tag="psT")
            nc.tensor.transpose(jt[:], ident[:], ident[:])

        # step-1 stationary slab + streamed A' on both queues + x^T slice
        st1 = stp.tile([128, KT, SW], F16, name="st1", tag="st")
        fA = fullp.tile([128, KT, G], F16, name="fA", tag="full")
        for k in range(KT):
            nc.sync.dma_start(st1[:, k, :], slabt0[128 * k : 128 * (k + 1), :])
            qeng[k % 2].dma_start(fA[:, k, :], afull[128 * k : 128 * (k + 1), :])
        xsb = misc.tile([128, KT, BW], F16, name="xsb", tag="xsb")
        for k in range(KT):
            qeng[k % 2].dma_start(xsb[:, k, :], xbj[128 * k : 128 * (k + 1), :])

        def product(st, prev_ssbs, F, si, reload_src=None, do_ag=False):
            """Emit one slab product = slab(st) @ F.

            If prev_ssbs is given, st is built chunk-by-chunk from them via
            TensorE transposes interleaved ahead of the consuming matmuls.
            """
            if reload_src is not None:
                for k in range(KT):
                    qeng[k % 2].dma_start(
                        F[:, k, :], reload_src[128 * k : 128 * (k + 1), :]
                    )
            ssbs = []
            for rnd in range(2):
                pts = [
                    [
                        mmps.tile(
                            [128, 512], F32, name=f"p{si}_{rnd}_{mr}_{mci}", tag="mm"
                        )
                        for mci in range(2)
                    ]
                    for mr in range(2)
                ]
                for k in range(KT):
                    if rnd == 0 and prev_ssbs is not None:
                        src = prev_ssbs[k // 8]
                        off = 128 * (k % 8)
                        for r in range(2):
                            psT = tps.tile(
                                [128, 128], F16, name=f"pt{si}_{k}_{r}", tag="psT"
                            )
                            nc.tensor.transpose(
                                psT[:], src[:, r, off : off + 128], ident[:]
                            )
                            nc.vector.tensor_copy(
                                st[:, k, 128 * r : 128 * (r + 1)], psT[:]
                            )
                    for mr in range(2):
                        for mci in range(2):
                            mc = 2 * rnd + mci
                            nc.tensor.matmul(
                                pts[mr][mci][:],
                                st[:, k, 128 * mr : 128 * (mr + 1)],
                                F[:, k, 512 * mc : 512 * (mc + 1)],
                                start=(k == 0),
                                stop=(k == KT - 1),
                            )
                ssb = ssp.tile([128, 2, _H], F16, name=f"ss{si}_{rnd}", tag="ss")
                for mr in range(2):
                    for mci in range(2):
                        nc.vector.tensor_copy(
                            ssb[:, mr, 512 * mci : 512 * (mci + 1)], pts[mr][mci][:]
                        )
                ssbs.append(ssb)
                if do_ag:
                    if rnd == 0:
                        agin = dram.tile([SW, G], F16, name=f"agin{si}", tag="agin")
                    for r in range(2):
                        nc.scalar.dma_start(
                            agin[128 * r : 128 * (r + 1), _H * rnd : _H * (rnd + 1)],
                            ssb[:, r, :],
                        )
            agout = None
            if do_ag:
                agout = dram.tile(
                    [G, G], F16, name=f"agout{si}", tag="agout", addr_space="Shared"
                )
                nc.gpsimd.collective_compute(
                    "AllGather",
                    mybir.AluOpType.bypass,
                    replica_groups=rg,
                    ins=[agin.opt()],
                    outs=[agout.opt()],
                )
            return ssbs, agout

        def new_st(si):
            return stp.tile([128, KT, SW], F16, name=f"st_{si}", tag="st")

        # chain
        ss, _ = product(st1, None, fA, "c2")
        st2 = new_st("c3")
        ss, _ = product(st2, ss, fA, "c3")
        st3 = new_st("c4")
        ss, ag4 = product(st3, ss, fA, "c4", do_ag=True)

        # filler C5 = C4 @ A' occupies the AG(C4) window and builds st4
        st4 = new_st("c8")
        ss, _ = product(st4, ss, fA, "f5")

        fC4 = fullp.tile([128, KT, G], F16, name="fC4", tag="full")
        ss, _ = product(st4, None, fC4, "c8", reload_src=ag4)
        st8 = new_st("c12")
        ss, ag12 = product(st8, ss, fC4, "c12", do_ag=True)

        # filler C13 = C12 @ A' occupies the AG(C12) window and builds st12
        st12 = new_st("c24")
        ss, _ = product(st12, ss, fA, "f13")

        fC12 = fullp.tile([128, KT, G], F16, name="fC12", tag="full")
        ss, _ = product(st12, None, fC12, "c24", reload_src=ag12)
        st24 = new_st("c36")
        ss, _ = product(st24, ss, fC12, "c36")
        st36 = new_st("c48")
        ss, ag48 = product(st36, ss, fC12, "c48", do_ag=True)

        # u^T = C12^T @ x^T during the AG(C48) window (C12 stationary)
        ut = misc.tile([128, KT, BW], F16, name="ut", tag="ut")
        for m in range(KT):
            pu = mmps.tile([128, BW], F32, name=f"pu{m}", tag="mm")
            for k in range(KT):
                nc.tensor.matmul(
                    pu[:],
                    fC12[:, k, 128 * m : 128 * (m + 1)],
                    xsb[:, k, :],
                    start=(k == 0),
                    stop=(k == KT - 1),
                )
            nc.vector.tensor_copy(ut[:, m, :], pu[:])

        # y[Bj,:] = u[Bj,:] @ C48
        fC48 = fullp.tile([128, KT, G], F16, name="fC48", tag="full")
        for k in range(KT):
            qeng[k % 2].dma_start(fC48[:, k, :], ag48[128 * k : 128 * (k + 1), :])
        for mr in range(4):
            pts = [
                mmps.tile([128, 512], F32, name=f"py{mr}_{mc}", tag="mm")
                for mc in range(4)
            ]
            for k in range(KT):
                for mc in range(4):
                    nc.tensor.matmul(
                        pts[mc][:],
                        ut[:, k, 128 * mr : 128 * (mr + 1)],
                        fC48[:, k, 512 * mc : 512 * (mc + 1)],
                        start=(k == 0),
                        stop=(k == KT - 1),
                    )
            for mc in range(4):
                yt = ypool.tile([128, 512], F32, name=f"yt{mr}_{mc}", tag="yt")
                nc.vector.tensor_copy(yt[:], pts[mc][:])
                qeng[mc % 2].dma_start(
                    yj[128 * mr : 128 * (mr + 1), 512 * mc : 512 * (mc + 1)], yt[:]
                )
    nc.compile()
    return nc


def _prep(x, W):
    """Host prep: rescale so the fp16 chain stays O(1); fp16 casts."""
    A = np.ascontiguousarray(W.T.astype(np.float64))
    rng = np.random.default_rng(0)
    v = rng.standard_normal(_G)
    growth = []
    for _ in range(60):
        v2 = A @ v
        n2 = np.linalg.norm(v2)
        growth.append(n2 / np.linalg.norm(v))
        v = v2 / n2
    rho = float(np.exp(np.mean(np.log(growth[20:]))))
    c = 1.0 / rho
    a16 = np.ascontiguousarray((c * A).astype(np.float16))
    x16t = np.ascontiguousarray(x.astype(np.float16).T)
    return a16, x16t, c


def kernel(x, W):
    from concourse.bass_utils import run_bass_kernel_spmd

    if "nc" not in _cache:
        _cache["nc"] = _build()
    nc = _cache["nc"]

    x = np.asarray(x, dtype=np.float32)
    W = np.asarray(W, dtype=np.float32)
    a16, x16t, c = _prep(x, W)
    in_maps = [
        {
            "afull": a16,
            "slabt0": np.ascontiguousarray(a16[_SW * j : _SW * (j + 1), :].T),
            "xbj": np.ascontiguousarray(x16t[:, _BW * j : _BW * (j + 1)]),
        }
        for j in range(_NC)
    ]
    res = run_bass_kernel_spmd(nc, in_maps, core_ids=list(range(_NC)))
    _cache["last_exec_time_ns"] = res.exec_time_ns
    _cache["last_results"] = res
    scale = np.float64(c) ** -60
    y = np.concatenate(
        [res.results[j]["yj"].astype(np.float64) * scale for j in range(_NC)], axis=0
    ).astype(np.float32)
    return y
